# revision 4
# baseline (speedup 1.0000x reference)
"""Trainium2 Bass kernel v2 for nn_AttentionBlock (B=16, C=512, H=W=32, 8 heads).

Data-parallel: 2 batches/core over 8 cores. Key techniques vs v1:
  - K=64 S-matmuls issued as adjacent row-half pairs -> 2x PE concurrency
    (measured 111.7 ns/MM vs 216 serial).
  - fp8e4 DoubleRow matmuls for QKV / V / AV / proj GEMMs (K=256 contraction
    per 213ns call = 2x bf16 FLOP rate). Weights scaled by 8 on host for fp8
    range; descale folded into exp-scale / finalize / output ops.
  - exp on ScalarE in [128,2048] chunks straight from one 4-bank PSUM tile,
    writing fp8 e tiles directly (exp arg shifted by -ln32 to stay in fp8
    range; softmax normalization cancels the shift).
  - AV is hf-major over persistent per-pc e tiles so only 2 acc PSUM banks
    are needed; denominators via interleaved ones columns in v2 (computed by
    the same AV matmuls), reciprocal + partition-shift SBUF DMA broadcast.
  - rstd = exp(-0.5*ln(var+eps)) keeps ScalarE on one ACT table set (no
    sqrt-set thrash); elementwise work split DVE/GpSimd.
"""

import math

import numpy as np
import ml_dtypes

import concourse.bass as bass
import concourse.bacc as bacc
import concourse.tile as tile
from concourse import mybir
from concourse.bass_utils import run_bass_kernel_spmd

P = 128
C = 512
T = 1024
N_HEADS = 8
HD = 64
B = 16
N_CORES = 8
B_LOC = B // N_CORES
CCH = C // P
EPS = 1e-5
WS = 8.0                      # fp8 weight scale
EB = -math.log(32.0)          # exp arg shift (cancels in softmax)
SC = 0.125 / (WS * WS)        # exp scale: 1/8 attn scale, /64 qk descale

F32 = mybir.dt.float32
BF16 = mybir.dt.bfloat16
FP8 = mybir.dt.float8e4
DR = mybir.MatmulPerfMode.DoubleRow

HALVES = ((0, slice(0, 512)), (1, slice(512, 1024)))


def _interleave(*seqs):
    items = []
    for si, s in enumerate(seqs):
        n = max(len(s), 1)
        for i, c in enumerate(s):
            items.append(((i + 0.5) / n, si, c))
    items.sort(key=lambda t: (t[0], t[1]))
    return [c for _, _, c in items]


def _emit(tc, nc, pools, aps, dbg=None):
    mul = mybir.AluOpType.mult
    add = mybir.AluOpType.add
    sub = mybir.AluOpType.subtract

    x_d, wqk_d, wv_d, wp_d, bqk_d, bv_d, bp_d, out_d = aps
    (const, xbp, x2p, xn8p, statp, xtmpp, qkp, v2p, ep, h8p, rdp, rd2p,
     hcpp, outp, pssp, accp, gpsp) = pools

    xv = x_d.rearrange("b (cc p) t -> b p cc t", p=P)
    ov = out_d.rearrange("b (cc p) t -> b p cc t", p=P)

    # ---- constants ----
    wqk_sb = const.tile([P, CCH, 2 * C], FP8)
    wv_sb = const.tile([P, CCH, C], FP8)
    wp_sb = const.tile([P, CCH, C], FP8)
    bqk_sb = const.tile([P, 8], F32)
    bp_sb = const.tile([P, CCH], F32)
    bv_b = const.tile([P, C], F32)
    ones_b = const.tile([P, P], BF16)
    eps_sb = const.tile([P, 1], F32)
    eb_sb = const.tile([P, 1], F32)

    def emit_consts():
        nc.vector.memset(ones_b, 1.0 / C)
        nc.vector.memset(eps_sb, EPS)
        nc.vector.memset(eb_sb, EB)
        nc.sync.dma_start(wqk_sb, wqk_d.rearrange("(cc p) o -> p cc o", p=P))
        nc.sync.dma_start(wv_sb, wv_d.rearrange("(cc p) o -> p cc o", p=P))
        nc.sync.dma_start(bqk_sb, bqk_d.rearrange("(o p) -> p o", p=P))
        nc.sync.dma_start(
            bv_b,
            bass.AP(tensor=bv_d.tensor, offset=bv_d.offset, ap=[[0, P]] + list(bv_d.ap)),
        )
        nc.sync.dma_start(bp_sb, bp_d.rearrange("(o p) -> p o", p=P))
        nc.sync.dma_start(wp_sb, wp_d.rearrange("(cc p) o -> p cc o", p=P))

    state = [dict() for _ in range(B_LOC)]

    # ---------------- phase A: LN + QKV + V ----------------
    def chunks_lnqkv(b):
        S = state[b]
        ch = []

        def c_load():
            S["xb"] = xbp.tile([P, CCH, T], BF16, tag="xb", name="xb_t")
            for cc in range(CCH):
                nc.sync.dma_start(S["xb"][:, cc], xv[b, :, cc])
            S["x2"] = x2p.tile([P, CCH, T], BF16, tag="x2", name="x2_t")
            S["v2"] = v2p.tile([P, N_HEADS, N_HEADS * P], FP8, tag="v2", name="v2_t")
            nc.gpsimd.memset(S["v2"], 1.0)

        ch.append(c_load)

        def c_xb(cc):
            nc.vector.tensor_tensor(S["x2"][:, cc], S["xb"][:, cc], S["xb"][:, cc], mul)

        for cc in range(CCH):
            ch.append(lambda cc=cc: c_xb(cc))

        def c_mu():
            S["m"] = statp.tile([P, T], F32, tag="stat", name="stat_t")
            for hf, hs in HALVES:
                ps = gpsp.tile([P, 512], F32, tag="gps", name="gps_t")
                for cc in range(CCH):
                    nc.tensor.matmul(
                        ps, ones_b, S["xb"][:, cc, hs], start=(cc == 0), stop=(cc == CCH - 1)
                    )
                nc.vector.tensor_copy(S["m"][:, hs], ps)

        ch.append(c_mu)

        def c_sq():
            S["var"] = statp.tile([P, T], F32, tag="stat", name="stat_t")
            S["e2"] = statp.tile([P, T], F32, tag="stat", name="stat_t")
            for hf, hs in HALVES:
                ps = gpsp.tile([P, 512], F32, tag="gps", name="gps_t")
                for cc in range(CCH):
                    nc.tensor.matmul(
                        ps, ones_b, S["x2"][:, cc, hs], start=(cc == 0), stop=(cc == CCH - 1)
                    )
                nc.vector.tensor_copy(S["e2"][:, hs], ps)

        ch.append(c_sq)

        def c_stats():
            nc.vector.tensor_tensor(S["var"], S["m"], S["m"], mul)
            nc.vector.tensor_tensor(S["var"], S["e2"], S["var"], sub)
            nc.scalar.activation(S["var"], S["var"], mybir.ActivationFunctionType.Ln, bias=eps_sb)
            rstd = statp.tile([P, T], F32, tag="stat", name="stat_t")
            nc.scalar.activation(rstd, S["var"], mybir.ActivationFunctionType.Exp, scale=-0.5)
            S["rstd"] = rstd
            S["xn"] = xn8p.tile([P, CCH, T], FP8, tag="xn8", name="xn8_t")

        ch.append(c_stats)

        def c_xn(cc):
            eng = nc.gpsimd if cc == 3 else nc.vector
            t = xtmpp.tile([P, T], F32, tag="xtmp", name="xtmp_t")
            eng.tensor_tensor(t, S["xb"][:, cc], S["m"], sub)
            eng.tensor_tensor(S["xn"][:, cc], t, S["rstd"], mul)

        for cc in range(CCH):
            ch.append(lambda cc=cc: c_xn(cc))

        def c_qk(ot):
            if "qk" not in S:
                S["qk"] = qkp.tile([P, N_HEADS, T], BF16, tag="qk", name="qk_t")
            for hf, hs in HALVES:
                ps = gpsp.tile([P, 512], F32, tag="gps", name="gps_t")
                for i in range(2):
                    nc.tensor.matmul(
                        ps,
                        wqk_sb[:, 2 * i : 2 * i + 2, ot * P : (ot + 1) * P],
                        S["xn"][:, 2 * i : 2 * i + 2, hs],
                        start=(i == 0),
                        stop=(i == 1),
                        perf_mode=DR,
                    )
                nc.vector.tensor_scalar_add(S["qk"][:, ot, hs], ps, bqk_sb[:, ot : ot + 1])

        for ot in range(8):
            ch.append(lambda ot=ot: c_qk(ot))

        def c_v(st):
            ps = gpsp.tile([P, 512], F32, tag="gps", name="gps_t")
            tsl = slice(st * P, (st + 1) * P)
            for i in range(2):
                nc.tensor.matmul(
                    ps,
                    S["xn"][:, 2 * i : 2 * i + 2, tsl],
                    wv_sb[:, 2 * i : 2 * i + 2, :],
                    start=(i == 0),
                    stop=(i == 1),
                    perf_mode=DR,
                )
            pr = ps.rearrange("p (h c) -> p h c", c=HD)
            bvr = bv_b.rearrange("p (h c) -> p h c", c=HD)
            v2r = S["v2"][:, st].rearrange("p (h c) -> p h c", c=P)
            nc.vector.tensor_tensor(v2r[:, 0::2, 0:HD], pr[:, 0::2], bvr[:, 0::2], add)
            nc.vector.tensor_tensor(v2r[:, 1::2, HD:P], pr[:, 1::2], bvr[:, 1::2], add)

        for st in range(8):
            ch.append(lambda st=st: c_v(st))

        def c_dbg():
            if dbg is not None and b == 0:
                nc.sync.dma_start(dbg["m"], S["m"])
                nc.sync.dma_start(dbg["rstd"], S["rstd"])
                nc.sync.dma_start(dbg["xn"], S["xn"])
                nc.sync.dma_start(dbg["qk"], S["qk"])
                nc.sync.dma_start(dbg["v2"], S["v2"])

        ch.append(c_dbg)
        return ch

    # ---------------- phase B: attention ----------------
    def chunks_attn(b):
        S = state[b]
        ch = []

        def c_pc_start(pc):
            S[("e", pc)] = ep.tile([P, N_HEADS, 2 * T], FP8, tag="e", name="e_t")
            if "h8" not in S:
                S["h8"] = h8p.tile([P, CCH, T], FP8, tag="h8", name="h8_t")

        def c_s(pc, st, h01):
            # per-h01 [128,1024] S-psum: double-buffers against the exp read so
            # ScalarE never bubbles; adjacent row-half MMs still pair on the PE.
            qt = S["qk"][:, 2 * pc]
            kt = S["qk"][:, 2 * pc + 1]
            tsl = slice(st * P, (st + 1) * P)
            bb = slice(HD * h01, HD * h01 + HD)
            pss = pssp.tile([P, T], F32, tag="pss", name="pss_t")
            for hf, hs in HALVES:
                nc.tensor.matmul(
                    pss[:, hs], kt[bb, tsl], qt[bb, hs], start=True, stop=True
                )
            nc.scalar.activation(
                S[("e", pc)][:, st, 1024 * h01 : 1024 * h01 + 1024], pss,
                mybir.ActivationFunctionType.Exp, scale=SC, bias=eb_sb,
            )

        def c_av(pc, hf, sp):
            if sp == 0:
                S[("acc", pc, hf)] = {}
            et = S[("e", pc)]
            for h01 in (0, 1):
                if sp == 0:
                    S[("acc", pc, hf)][h01] = accp.tile([P, 512], F32, tag="acc", name="acc_t")
                head = 2 * pc + h01
                fs = 1024 * h01 + 512 * hf
                nc.tensor.matmul(
                    S[("acc", pc, hf)][h01],
                    S["v2"][:, 2 * sp : 2 * sp + 2, head * P : (head + 1) * P],
                    et[:, 2 * sp : 2 * sp + 2, fs : fs + 512],
                    start=(sp == 0),
                    stop=(sp == 3),
                    perf_mode=DR,
                )

        def c_fin(pc, hf, h01):
            hs = HALVES[hf][1]
            acc = S[("acc", pc, hf)][h01]
            data = slice(HD * h01, HD * h01 + HD)
            dnm = slice(HD * (1 - h01), HD * (1 - h01) + HD)
            # copy acc out of PSUM first so the accumulator bank frees fast
            # (next hf-pass AV matmuls only wait ~0.7us, not the DMA chain).
            hcp = hcpp.tile([P, 512], F32, tag="hcp", name="hcp_t")
            nc.vector.tensor_copy(hcp, acc)
            # reciprocal only ever on base-partition-0 regions (DVE quirk);
            # denominator row makes a DRAM round-trip to broadcast across the
            # data partitions (SBUF->SBUF partition-shift DMA races).
            rd = rdp.tile([P, 512], F32, tag="rd", name="rd_t")
            sc = rd2p.tile([1, 512], F32, tag="rdd", name="rdd_t")
            if h01 == 1:
                nc.vector.reciprocal_approx_fast(rd[dnm], hcp[dnm])
                nc.sync.dma_start(sc, rd[dnm.start : dnm.start + 1, :])
            else:
                nc.sync.dma_start(sc, hcp[dnm.start : dnm.start + 1, :])
            bcast = bass.AP(
                tensor=sc.tensor, offset=sc.offset,
                ap=[[0, HD]] + [list(a) for a in sc.ap[1:]],
            )
            nc.sync.dma_start(rd[data], bcast)
            if h01 == 0:
                nc.vector.reciprocal_approx_fast(rd[data], rd[data])
            # last pc of b1: DVE is idle in the tail, gpsimd queue is not
            meng = nc.vector if (b == 1 and pc == 3) else nc.gpsimd
            meng.tensor_tensor(S["h8"][data, pc, hs], hcp[data], rd[data], mul)

        # AV work for pc is delayed one pc and interleaved into pc+1's S
        # stream: by then every e-plane it reads exists, so its matmuls enter
        # the PE FIFO with satisfied deps and never head-of-line-block the S
        # matmuls that feed ScalarE (the bottleneck engine).
        def stream(pc):
            sch = [lambda pc=pc: c_pc_start(pc)]
            for st in range(8):
                sch.append(lambda pc=pc, st=st: c_s(pc, st, 0))
                sch.append(lambda pc=pc, st=st: c_s(pc, st, 1))
            return sch

        def avwork(pc):
            ach = []
            for sp in range(4):
                ach.append(lambda pc=pc, sp=sp: c_av(pc, 0, sp))
            ach.append(lambda pc=pc: c_fin(pc, 0, 0))
            ach.append(lambda pc=pc: c_fin(pc, 0, 1))
            for sp in range(4):
                ach.append(lambda pc=pc, sp=sp: c_av(pc, 1, sp))
            ach.append(lambda pc=pc: c_fin(pc, 1, 0))
            ach.append(lambda pc=pc: c_fin(pc, 1, 1))
            return ach

        ch.extend(stream(0))
        for pc in range(1, 4):
            ch.extend(_interleave(stream(pc), avwork(pc - 1)))
        ch.extend(avwork(3))

        def c_dbg():
            if dbg is not None and b == 0:
                nc.sync.dma_start(dbg["e0"], S[("e", 0)])
                nc.sync.dma_start(dbg["h8"], S["h8"])

        ch.append(c_dbg)
        return ch

    # ---------------- phase C: proj + residual ----------------
    def chunks_proj(b):
        S = state[b]
        ch = []

        def c_pj(ot, hf, hs):
            ps = gpsp.tile([P, 512], F32, tag="gps", name="gps_t")
            for i in range(2):
                nc.tensor.matmul(
                    ps,
                    wp_sb[:, 2 * i : 2 * i + 2, ot * P : (ot + 1) * P],
                    S["h8"][:, 2 * i : 2 * i + 2, hs],
                    start=(i == 0),
                    stop=(i == 1),
                    perf_mode=DR,
                )
            o_t = outp.tile([P, 512], F32, tag="out", name="out_t")
            nc.vector.tensor_scalar(
                o_t, ps, 1.0 / (WS * WS), bp_sb[:, ot : ot + 1], mul, add
            )
            # spread residual adds across both elementwise engines
            eng = nc.gpsimd if (b == 0 or (ot + hf) % 2 == 0) else nc.vector
            eng.tensor_tensor(o_t, o_t, S["xb"][:, ot, hs], add)
            nc.sync.dma_start(ov[b, :, ot, hs], o_t)

        for ot in range(CCH):
            for hf, hs in HALVES:
                ch.append(lambda ot=ot, hf=hf, hs=hs: c_pj(ot, hf, hs))
        return ch

    # ---------------- emission schedule ----------------
    # b0 prologue runs through stats/xn and the first two qk blocks (enough
    # for pc0's S matmuls); the remaining b0 v/qk chunks are emitted right
    # after the first S/exp pair but BEFORE any AV chunk (deps only track
    # reads-after-previously-emitted-writes).
    a0 = chunks_lnqkv(0)
    n_pre = 14  # load, xb x4, mu, sq, stats, xn x4, qk0, qk1
    a0[0]()
    emit_consts()
    for c in a0[1:n_pre]:
        c()
    at0 = chunks_attn(0)
    n_s0 = 17  # stream(0): pc_start + 16 S chunks, no AV readers inside
    # a0's remaining qk/v chunks merge into stream(0) and are all emitted
    # before stream(1), whose interleaved avwork(0) reads v2/qk.
    for c in _interleave(at0[:n_s0], a0[n_pre:]):
        c()
    for c in _interleave(at0[n_s0:], chunks_lnqkv(1)):
        c()
    for c in _interleave(chunks_attn(1), chunks_proj(0)):
        c()
    for c in chunks_proj(1):
        c()


def build_nc(debug_taps=False):
    nc = bacc.Bacc("TRN2", num_devices=N_CORES, debug=False)
    x = nc.declare_dram_parameter("xbh", [B_LOC, C, T], BF16, isOutput=False)
    wqk = nc.declare_dram_parameter("w_qkT8", [C, 2 * C], FP8, isOutput=False)
    wv = nc.declare_dram_parameter("w_vT8", [C, C], FP8, isOutput=False)
    wp = nc.declare_dram_parameter("w_projT8", [C, C], FP8, isOutput=False)
    bqk = nc.declare_dram_parameter("b_qk8", [2 * C], F32, isOutput=False)
    bv = nc.declare_dram_parameter("b_v8", [C], F32, isOutput=False)
    bp = nc.declare_dram_parameter("b_proj", [C], F32, isOutput=False)
    out = nc.declare_dram_parameter("out", [B_LOC, C, T], F32, isOutput=True)
    aps = (x.ap(), wqk.ap(), wv.ap(), wp.ap(), bqk.ap(), bv.ap(), bp.ap(), out.ap())
    dbg = None
    if debug_taps:
        dbg = {
            "m": nc.declare_dram_parameter("dbg_m", [P, T], F32, isOutput=True).ap(),
            "rstd": nc.declare_dram_parameter("dbg_rstd", [P, T], F32, isOutput=True).ap(),
            "xn": nc.declare_dram_parameter("dbg_xn", [P, CCH, T], BF16, isOutput=True).ap(),
            "qk": nc.declare_dram_parameter("dbg_qk", [P, N_HEADS, T], BF16, isOutput=True).ap(),
            "v2": nc.declare_dram_parameter("dbg_v2", [P, N_HEADS, N_HEADS * P], FP8, isOutput=True).ap(),
            "e0": nc.declare_dram_parameter("dbg_e0", [P, N_HEADS, 2 * T], FP8, isOutput=True).ap(),
            "h8": nc.declare_dram_parameter("dbg_h8", [P, CCH, T], FP8, isOutput=True).ap(),
        }

    with tile.TileContext(nc) as tc:
        import contextlib

        with contextlib.ExitStack() as ctx:
            pools = (
                ctx.enter_context(tc.tile_pool(name="const", bufs=1)),
                ctx.enter_context(tc.tile_pool(name="xb", bufs=2)),
                ctx.enter_context(tc.tile_pool(name="x2", bufs=1)),
                ctx.enter_context(tc.tile_pool(name="xn8", bufs=2)),
                ctx.enter_context(tc.tile_pool(name="stat", bufs=4)),
                ctx.enter_context(tc.tile_pool(name="xtmp", bufs=2)),
                ctx.enter_context(tc.tile_pool(name="qk", bufs=2)),
                ctx.enter_context(tc.tile_pool(name="v2", bufs=2)),
                ctx.enter_context(tc.tile_pool(name="e", bufs=2)),
                ctx.enter_context(tc.tile_pool(name="h8", bufs=2)),
                ctx.enter_context(tc.tile_pool(name="rd", bufs=4)),
                ctx.enter_context(tc.tile_pool(name="rd2", bufs=8, space="DRAM")),
                ctx.enter_context(tc.tile_pool(name="hcp", bufs=4)),
                ctx.enter_context(tc.tile_pool(name="out", bufs=3)),
                ctx.enter_context(tc.tile_pool(name="pss", bufs=2, space="PSUM")),
                ctx.enter_context(tc.tile_pool(name="acc", bufs=2, space="PSUM")),
                ctx.enter_context(tc.tile_pool(name="gps", bufs=2, space="PSUM")),
            )
            _emit(tc, nc, pools, aps, dbg)
    nc.compile()
    return nc


def _host_prep(w_qkv, b_qkv, w_proj, b_proj):
    rows = np.arange(3 * C).reshape(N_HEADS, 3, HD)
    qk_order = []
    for pc in range(4):
        qk_order += list(rows[2 * pc, 0]) + list(rows[2 * pc + 1, 0])
        qk_order += list(rows[2 * pc, 1]) + list(rows[2 * pc + 1, 1])
    qk_order = np.array(qk_order)
    v_order = rows[:, 2, :].reshape(-1)
    f8 = ml_dtypes.float8_e4m3fn
    prep = {
        "w_qkT8": np.ascontiguousarray((w_qkv[qk_order] * WS).T).astype(f8),
        "w_vT8": np.ascontiguousarray((w_qkv[v_order] * WS).T).astype(f8),
        "w_projT8": np.ascontiguousarray((w_proj * WS).T).astype(f8),
        "b_qk8": np.ascontiguousarray(b_qkv[qk_order] * WS).astype(np.float32),
        "b_v8": np.ascontiguousarray(b_qkv[v_order] * WS).astype(np.float32),
        "b_proj": np.ascontiguousarray(b_proj).astype(np.float32),
    }
    return prep


_NC = None


def kernel(x, emb, w_qkv, b_qkv, w_proj, b_proj):
    global _NC
    x = np.asarray(x, dtype=np.float32)
    b, c, hh, ww = x.shape
    assert (b, c, hh * ww) == (B, C, T)
    prep = _host_prep(
        np.asarray(w_qkv, np.float32),
        np.asarray(b_qkv, np.float32),
        np.asarray(w_proj, np.float32),
        np.asarray(b_proj, np.float32),
    )
    xf = x.reshape(B, C, T)
    if _NC is None:
        _NC = build_nc()
    in_maps = []
    for core in range(N_CORES):
        m = dict(prep)
        m["xbh"] = np.ascontiguousarray(
            xf[core * B_LOC : (core + 1) * B_LOC]
        ).astype(ml_dtypes.bfloat16)
        in_maps.append(m)
    res = run_bass_kernel_spmd(_NC, in_maps, core_ids=list(range(N_CORES)), trace=False)
    out = np.concatenate([res.results[i]["out"] for i in range(N_CORES)], axis=0)
    return out.reshape(B, C, hh, ww).astype(np.float32)
